# revision 7
# baseline (speedup 1.0000x reference)
"""GRU image-caption decoder on 8 Trainium2 NeuronCores.

Problem: B=128, T=24, E=H=512, V=12000.
  x_cat = [img, emb[cap[:, :-1]]]                  # [B, T, E]
  gx    = x_cat @ W_ih.T  (+ b_ih == 0)            # [B, T, 3H]
  h_{t+1} = GRU-step(h_t, gx_t)  (b_hh == 0)       # 24 serial steps
  logits  = hs @ W_out.T + b_out                   # [B, T, V]

Sharding: pure data-parallel over batch, 16 rows per core.  Each core
runs the full pipeline for its batch shard; no collectives.  Rows on
device are t-major (row = t*16 + b) so each GRU step's gx slice and each
classifier M-tile (128 rows = 8 steps) is contiguous.

On-device layout choices:
  - gx GEMM in fp32 (float32r PE mode), bounced through DRAM so per-step
    [16, 3H] slices land at partition 0 (engines only accept a limited
    set of start partitions).
  - Recurrence: gh = h @ W_hh.T as out[16, 3H] with stationary lhsT =
    hT[128, 16] slices of the bf16 hsT stash; W_hh.T streams (bf16).
    Gate math in fp32 on [16, 256] half-tiles spread across DVE/ACT/GPS.
  - h' is PE-transposed ([16,128] -> [128,16]) into the bf16 hsT stash,
    which doubles as the classifier lhsT (M-tiles of 128 rows).
  - Classifier: hsT-block @ W_out.T (bf16) in 24 column chunks of 500;
    bias + PSUM evacuation fused in one scalar_tensor_tensor on DVE.
"""

import os
import sys

if "/opt/trn_rl_repo" not in sys.path:
    sys.path.insert(0, "/opt/trn_rl_repo")

import numpy as np
import ml_dtypes
from contextlib import ExitStack

import concourse.bass as bass
import concourse.bacc as bacc
import concourse.mybir as mybir
import concourse.tile as tile
from concourse.bass_utils import run_bass_kernel_spmd

F32 = mybir.dt.float32
F32R = mybir.dt.float32r
BF16 = mybir.dt.bfloat16
AF = mybir.ActivationFunctionType
ALU = mybir.AluOpType

B, T, E, H, V = 128, 24, 512, 512, 12000
NCORES = 8
BC = B // NCORES          # 16 batch rows per core
R = BC * T                # 384 on-device rows, t-major
G3 = 3 * H                # 1536
KT = H // 128             # 4 contraction tiles
NCH = 24                  # classifier column chunks
CW = V // NCH             # 500 columns per chunk
NG = R // 128             # 3 classifier M-tiles (each 8 steps)

_CACHE = {}
LAST_RESULTS = None       # test.py reads profiling info from here


def _build():
    nc = bacc.Bacc("TRN2", target_bir_lowering=False, debug=False)

    xT = nc.dram_tensor("xT", [E, R], BF16, kind="ExternalInput")
    wihT = nc.dram_tensor("wihT", [E, G3], BF16, kind="ExternalInput")
    whhT = nc.dram_tensor("whhT", [H, G3], BF16, kind="ExternalInput")
    woutT = nc.dram_tensor("woutT", [H, V], BF16, kind="ExternalInput")
    boutr = nc.dram_tensor("boutr", [128, V], F32, kind="ExternalInput")
    ident = nc.dram_tensor("ident", [16, 16], F32, kind="ExternalInput")
    out = nc.dram_tensor("out", [R, V], F32, kind="ExternalOutput")

    with tile.TileContext(nc) as tc, ExitStack() as ctx:
        wpool = ctx.enter_context(tc.tile_pool(name="w", bufs=1))
        state = ctx.enter_context(tc.tile_pool(name="state", bufs=1))
        work = ctx.enter_context(tc.tile_pool(name="work", bufs=1))
        gxp = ctx.enter_context(tc.tile_pool(name="gxp", bufs=2))
        outp = ctx.enter_context(tc.tile_pool(name="outp", bufs=4))
        dram = ctx.enter_context(tc.tile_pool(name="dram", bufs=1, space="DRAM"))
        psA = ctx.enter_context(tc.tile_pool(name="psA", bufs=1, space="PSUM"))
        psB = ctx.enter_context(tc.tile_pool(name="psB", bufs=2, space="PSUM"))
        psC = ctx.enter_context(tc.tile_pool(name="psC", bufs=2, space="PSUM"))

        # ---------------- phase 1: gx = x_cat @ W_ih.T -> DRAM bounce ------
        gx_d = dram.tile([R, G3], F32, tag="gxd")
        with tc.tile_pool(name="p1", bufs=1) as p1, \
             tc.tile_pool(name="p1s", bufs=1) as p1s:
            xT_t = []
            wih_t = []
            for k in range(KT):
                xt = p1.tile([128, R], BF16, tag=f"xT{k}")
                nc.sync.dma_start(xt[:], xT[k * 128:(k + 1) * 128, :])
                xT_t.append(xt)
            for k in range(KT):
                wt = p1.tile([128, G3], BF16, tag=f"wih{k}")
                nc.sync.dma_start(wt[:], wihT[k * 128:(k + 1) * 128, :])
                wih_t.append(wt)
            for m in range(NG):
                p = psA.tile([128, G3], F32, tag="gh")
                for nch in range(3):
                    csl = slice(nch * 512, (nch + 1) * 512)
                    for k in range(KT):
                        nc.tensor.matmul(
                            p[:, csl],
                            xT_t[k][:, m * 128:(m + 1) * 128],
                            wih_t[k][:, csl],
                            start=(k == 0), stop=(k == KT - 1),
                        )
                s = p1s.tile([128, G3], F32, tag="gxs")
                nc.scalar.copy(s[:], p[:])
                nc.sync.dma_start(gx_d[m * 128:(m + 1) * 128, :], s[:])

        # ---------------- resident weights ---------------------------------
        whh_t = []
        for k in range(KT):
            wt = wpool.tile([128, G3], BF16, tag=f"whh{k}")
            nc.sync.dma_start(wt[:], whhT[k * 128:(k + 1) * 128, :])
            whh_t.append(wt)
        id_t = wpool.tile([16, 16], F32, tag="id")
        nc.sync.dma_start(id_t[:], ident[:])
        wout_t = []
        for k in range(KT):
            wt = wpool.tile([128, V], BF16, tag=f"wout{k}")
            nc.sync.dma_start(wt[:], woutT[k * 128:(k + 1) * 128, :])
            wout_t.append(wt)
        # hsT stash: h_{t+1} lives at group g = t // 8, cols (t % 8) * 16.
        # [KT][NG] tiles so classifier deps attach per group, not per stash.
        hsT = [[state.tile([128, 128], BF16, tag=f"hsT{k}_{g}",
                           name=f"hsT{k}_{g}")
                for g in range(NG)] for k in range(KT)]

        # classifier unit (g, ch)
        def cls_unit(g, ch):
            csl = slice(ch * CW, (ch + 1) * CW)
            p = psB.tile([128, CW], F32, tag="cls")
            for k in range(KT):
                nc.tensor.matmul(
                    p[:], hsT[k][g][:], wout_t[k][:, csl],
                    start=(k == 0), stop=(k == KT - 1),
                )
            bt = outp.tile([128, CW], F32, tag="bct")
            nc.sync.dma_start(bt[:], boutr[:, csl])
            o = outp.tile([128, CW], F32, tag="ostage")
            nc.vector.scalar_tensor_tensor(
                o[:], p[:], 1.0, bt[:], op0=ALU.mult, op1=ALU.add)
            nc.sync.dma_start(out[g * 128:(g + 1) * 128, csl], o[:])

        cls_units = [(g, ch) for g in range(NG) for ch in range(NCH)]
        cls_done = 0

        # ---------------- recurrence ---------------------------------------
        h_prev = None  # A-layout [16, 512] f32 tile of h_t
        for t in range(T):
            gx_t = gxp.tile([BC, G3], F32, tag="gxt")
            nc.sync.dma_start(gx_t[:], gx_d[t * BC:(t + 1) * BC, :])

            if t > 0:
                g_prev, s_prev = (t - 1) // 8, (t - 1) % 8
                p_gh = psA.tile([128, G3], F32, tag="gh")
                for nch in range(3):
                    csl = slice(nch * 512, (nch + 1) * 512)
                    for k in range(KT):
                        nc.tensor.matmul(
                            p_gh[0:BC, csl],
                            hsT[k][g_prev][:, s_prev * 16:(s_prev + 1) * 16],
                            whh_t[k][:, csl],
                            start=(k == 0), stop=(k == KT - 1),
                        )

            h_new = work.tile([BC, H], F32, tag="hA", bufs=2)
            for c in range(2):
                sl = slice(c * 256, (c + 1) * 256)
                xr = gx_t[:, c * 256:(c + 1) * 256]
                xz = gx_t[:, 512 + c * 256:512 + (c + 1) * 256]
                xn = gx_t[:, 1024 + c * 256:1024 + (c + 1) * 256]

                r = work.tile([BC, 256], F32, tag=f"r{c}", bufs=2)
                z = work.tile([BC, 256], F32, tag=f"z{c}", bufs=2)
                n = work.tile([BC, 256], F32, tag=f"n{c}", bufs=2)
                omz = work.tile([BC, 256], F32, tag=f"omz{c}", bufs=2)

                if t == 0:
                    nc.scalar.activation(r[:], xr, AF.Sigmoid)
                    nc.scalar.activation(z[:], xz, AF.Sigmoid)
                    nc.scalar.activation(n[:], xn, AF.Tanh)
                    # h1 = (1 - z) * n
                    nc.vector.tensor_scalar(
                        omz[:], z[:], -1.0, 1.0, op0=ALU.mult, op1=ALU.add)
                    nc.vector.tensor_tensor(
                        h_new[:, sl], omz[:], n[:], op=ALU.mult)
                else:
                    hr = p_gh[0:BC, c * 256:(c + 1) * 256]
                    hz = p_gh[0:BC, 512 + c * 256:512 + (c + 1) * 256]
                    hn = p_gh[0:BC, 1024 + c * 256:1024 + (c + 1) * 256]

                    # GpSimd cannot read PSUM, so every op touching p_gh
                    # stays on DVE; SBUF-only side computations go to GpSimd.
                    rp = work.tile([BC, 256], F32, tag=f"rp{c}")
                    nc.vector.tensor_tensor(rp[:], hr, xr, op=ALU.add)
                    nc.scalar.activation(r[:], rp[:], AF.Sigmoid)

                    zp = work.tile([BC, 256], F32, tag=f"zp{c}")
                    nc.vector.tensor_tensor(zp[:], hz, xz, op=ALU.add)
                    nc.scalar.activation(z[:], zp[:], AF.Sigmoid)

                    rhn = work.tile([BC, 256], F32, tag=f"rhn{c}")
                    nc.vector.tensor_tensor(rhn[:], r[:], hn, op=ALU.mult)
                    nc.vector.tensor_tensor(rhn[:], rhn[:], xn, op=ALU.add)
                    nc.scalar.activation(n[:], rhn[:], AF.Tanh)

                    nc.gpsimd.tensor_scalar(
                        omz[:], z[:], -1.0, 1.0, op0=ALU.mult, op1=ALU.add)
                    zh = work.tile([BC, 256], F32, tag=f"zh{c}")
                    nc.gpsimd.tensor_tensor(
                        zh[:], z[:], h_prev[:, sl], op=ALU.mult)
                    omzn = work.tile([BC, 256], F32, tag=f"omzn{c}")
                    nc.vector.tensor_tensor(omzn[:], omz[:], n[:], op=ALU.mult)
                    nc.vector.tensor_tensor(
                        h_new[:, sl], omzn[:], zh[:], op=ALU.add)

            # transpose h_{t+1} into the bf16 hsT stash
            g, s = t // 8, t % 8
            p_tr = psC.tile([128, 64], F32, tag="tr")
            for hc in range(KT):
                nc.tensor.transpose(
                    p_tr[:, hc * 16:(hc + 1) * 16],
                    h_new[:, hc * 128:(hc + 1) * 128], id_t[:])
            for hc in range(KT):
                nc.scalar.copy(
                    hsT[hc][g][:, s * 16:(s + 1) * 16],
                    p_tr[:, hc * 16:(hc + 1) * 16])
            h_prev = h_new

            # interleave classifier work once its M-tile group is complete;
            # pace ~3 units/step so cls matmuls fill PE gaps in the serial
            # recurrence instead of hogging the PE ahead of spine matmuls
            if t >= 7:
                avail = 24 * ((t + 1) // 8)
                target = min(avail, 3 * (t - 6))
                while cls_done < target:
                    cls_unit(*cls_units[cls_done])
                    cls_done += 1
        while cls_done < len(cls_units):
            cls_unit(*cls_units[cls_done])
            cls_done += 1

    nc.compile()
    return nc


def _prep(inputs):
    img = np.asarray(inputs["img"], np.float32)
    cap = np.asarray(inputs["cap"], np.int64)
    emb = np.asarray(inputs["emb"], np.float32)
    W_ih = np.asarray(inputs["W_ih"], np.float32)
    W_hh = np.asarray(inputs["W_hh"], np.float32)
    W_out = np.asarray(inputs["W_out"], np.float32)
    b_out = np.asarray(inputs["b_out"], np.float32)
    # b_ih / b_hh are structurally zero in this problem's setup_inputs.

    word = emb[cap[:, :-1]]                       # [B, T-1, E]
    x = np.concatenate([img[:, None, :], word], axis=1)  # [B, T, E]

    wihT = np.ascontiguousarray(W_ih.T).astype(ml_dtypes.bfloat16)
    whhT = np.ascontiguousarray(W_hh.T).astype(ml_dtypes.bfloat16)
    woutT = np.ascontiguousarray(W_out.T).astype(ml_dtypes.bfloat16)
    boutr = np.ascontiguousarray(np.broadcast_to(b_out, (128, V)))
    id16 = np.eye(16, dtype=np.float32)

    in_maps = []
    for c in range(NCORES):
        xc = x[c * BC:(c + 1) * BC]               # [16, T, E]
        xTc = np.ascontiguousarray(
            xc.transpose(2, 1, 0).reshape(E, R)).astype(ml_dtypes.bfloat16)
        in_maps.append({
            "xT": xTc, "wihT": wihT, "whhT": whhT, "woutT": woutT,
            "boutr": boutr, "ident": id16,
        })
    return in_maps


def run_spmd(in_maps):
    """Compile (cached) + execute the SPMD program; returns BassKernelResults."""
    if "nc" not in _CACHE:
        _CACHE["nc"] = _build()
    return run_bass_kernel_spmd(_CACHE["nc"], in_maps, list(range(NCORES)))


def kernel(**inputs):
    global LAST_RESULTS
    in_maps = _prep(inputs)
    res = run_spmd(in_maps)
    LAST_RESULTS = res
    logits = np.empty((B, T, V), np.float32)
    for c in range(NCORES):
        o = res.results[c]["out"]                 # [R, V], t-major rows
        logits[c * BC:(c + 1) * BC] = o.reshape(T, BC, V).transpose(1, 0, 2)
    return logits


# revision 17
# speedup vs baseline: 19352.7077x; 19352.7077x over previous
"""GRU image-caption decoder on 8 Trainium2 NeuronCores.

Problem: B=128, T=24, E=H=512, V=12000.
  x_cat = [img, emb[cap[:, :-1]]]                  # [B, T, E]
  gx    = x_cat @ W_ih.T  (+ b_ih == 0)            # [B, T, 3H]
  h_{t+1} = GRU-step(h_t, gx_t)  (b_hh == 0)       # 24 serial steps
  logits  = hs @ W_out.T + b_out                   # [B, T, V]

Sharding: pure data-parallel over batch, 16 rows per core.  Each core
runs the full pipeline for its batch shard; no collectives.  Rows on
device are t-major (row = t*16 + b) so each GRU step's gx slice and each
classifier M-tile (128 rows = 8 steps) is contiguous.

On-device layout choices:
  - gx GEMM in fp32 (float32r PE mode), bounced through DRAM so per-step
    [16, 3H] slices land at partition 0 (engines only accept a limited
    set of start partitions).
  - Recurrence: gh = h @ W_hh.T as out[16, 3H] with stationary lhsT =
    hT[128, 16] slices of the bf16 hsT stash; W_hh.T streams (bf16).
    Gate math in fp32 on [16, 256] half-tiles spread across DVE/ACT/GPS.
  - h' is PE-transposed ([16,128] -> [128,16]) into the bf16 hsT stash,
    which doubles as the classifier lhsT (M-tiles of 128 rows).
  - Classifier: hsT-block @ W_out.T (bf16) in 24 column chunks of 500;
    bias + PSUM evacuation fused in one scalar_tensor_tensor on DVE.
"""

import os
import sys

if "/opt/trn_rl_repo" not in sys.path:
    sys.path.insert(0, "/opt/trn_rl_repo")

import numpy as np
import ml_dtypes
from contextlib import ExitStack

import concourse.bass as bass
import concourse.bacc as bacc
import concourse.mybir as mybir
import concourse.tile as tile
from concourse.bass_utils import run_bass_kernel_spmd

F32 = mybir.dt.float32
F32R = mybir.dt.float32r
BF16 = mybir.dt.bfloat16
AF = mybir.ActivationFunctionType
ALU = mybir.AluOpType

B, T, E, H, V = 128, 24, 512, 512, 12000
NCORES = 8
BC = B // NCORES          # 16 batch rows per core
R = BC * T                # 384 on-device rows, t-major
G3 = 3 * H                # 1536
KT = H // 128             # 4 contraction tiles
NCH = 24                  # classifier column chunks
CW = V // NCH             # 500 columns per chunk
NG = R // 128             # 3 classifier M-tiles (each 8 steps)

_CACHE = {}
LAST_RESULTS = None       # test.py reads profiling info from here


def _build(loop_reps=0):
    nc = bacc.Bacc("TRN2", target_bir_lowering=False, debug=False)

    xT = nc.dram_tensor("xT", [E, R], BF16, kind="ExternalInput")
    wihT = nc.dram_tensor("wihT", [E, G3], BF16, kind="ExternalInput")
    whhT = nc.dram_tensor("whhT", [H, G3], BF16, kind="ExternalInput")
    woutT = nc.dram_tensor("woutT", [H, V], BF16, kind="ExternalInput")
    boutr = nc.dram_tensor("boutr", [1, V], BF16, kind="ExternalInput")
    ident = nc.dram_tensor("ident", [16, 16], F32, kind="ExternalInput")
    out = nc.dram_tensor("out", [R, V], F32, kind="ExternalOutput")

    with tile.TileContext(nc) as tc, ExitStack() as ctx:
        wpool = ctx.enter_context(tc.tile_pool(name="w", bufs=1))
        state = ctx.enter_context(tc.tile_pool(name="state", bufs=1))
        work = ctx.enter_context(tc.tile_pool(name="work", bufs=1))
        gxp = ctx.enter_context(tc.tile_pool(name="gxp", bufs=2))
        outp = ctx.enter_context(tc.tile_pool(name="outp", bufs=4))
        dram = ctx.enter_context(tc.tile_pool(name="dram", bufs=1, space="DRAM"))
        psA = ctx.enter_context(tc.tile_pool(name="psA", bufs=1, space="PSUM"))
        psB = ctx.enter_context(tc.tile_pool(name="psB", bufs=3, space="PSUM"))
        psC = ctx.enter_context(tc.tile_pool(name="psC", bufs=2, space="PSUM"))

        # ---------------- phase 1: gx = x_cat @ W_ih.T -> DRAM bounce ------
        import contextlib
        loop_cm = tc.For_i(0, loop_reps, 1) if loop_reps else \
            contextlib.nullcontext()
        gx_d = [dram.tile([128, G3], BF16, tag=f"gxd{m}",
                          name=f"gxd{m}") for m in range(NG)]
        ctx.enter_context(loop_cm)
        with tc.tile_pool(name="p1", bufs=1) as p1, \
             tc.tile_pool(name="p1s", bufs=1) as p1s:
            xT_t = []
            wih_t = []
            for k in range(KT):
                xt = p1.tile([128, R], BF16, tag=f"xT{k}", name=f"xt{k}")
                nc.sync.dma_start(xt[:], xT[k * 128:(k + 1) * 128, :])
                xT_t.append(xt)
                wt = p1.tile([128, G3], BF16, tag=f"wih{k}", name=f"wiht{k}")
                nc.sync.dma_start(wt[:], wihT[k * 128:(k + 1) * 128, :])
                wih_t.append(wt)
            for m in range(NG):
                p = psA.tile([128, G3], F32, tag="gh")
                for nch in range(3):
                    csl = slice(nch * 512, (nch + 1) * 512)
                    for k in range(KT):
                        nc.tensor.matmul(
                            p[:, csl],
                            xT_t[k][:, m * 128:(m + 1) * 128],
                            wih_t[k][:, csl],
                            start=(k == 0), stop=(k == KT - 1),
                        )
                s = p1s.tile([128, G3], BF16, tag="gxs")
                nc.scalar.copy(s[:], p[:])
                nc.sync.dma_start(gx_d[m][:], s[:])

        # ---------------- resident weights ---------------------------------
        whh_t = []
        for k in range(KT):
            whhsb = wpool.tile([128, G3], BF16, tag=f"whh{k}", name=f"whhsb{k}")
            nc.sync.dma_start(whhsb[:], whhT[k * 128:(k + 1) * 128, :])
            whh_t.append(whhsb)
        id_t = wpool.tile([16, 16], F32, tag="id")
        nc.sync.dma_start(id_t[:], ident[:])
        bout_sb = wpool.tile([1, V], BF16, tag="bout")
        ones_t = wpool.tile([1, 128], BF16, tag="ones")
        wout_t = [wpool.tile([128, V], BF16, tag=f"wout{k}", name=f"woutsb{k}")
                  for k in range(KT)]

        wout_dma_batches = []
        for ch in range(NCH):
            csl = slice(ch * CW, (ch + 1) * CW)
            for k in range(KT):
                wout_dma_batches.append((k, csl))

        def emit_wout_dmas(lo, hi):
            for k, csl in wout_dma_batches[lo:hi]:
                nc.sync.dma_start(
                    wout_t[k][:, csl], woutT[k * 128:(k + 1) * 128, csl])
        # hsT stash: h_{t+1} lives at group g = t // 8, cols (t % 8) * 16.
        # [KT][NG] tiles so classifier deps attach per group, not per stash.
        hsT = [[state.tile([128, 128], BF16, tag=f"hsT{k}_{g}",
                           name=f"hsT{k}_{g}")
                for g in range(NG)] for k in range(KT)]

        # classifier unit (g, ch)
        def cls_unit(g, ch):
            csl = slice(ch * CW, (ch + 1) * CW)
            p = psB.tile([128, CW], F32, tag="cls")
            for k in range(KT):
                nc.tensor.matmul(
                    p[:], hsT[k][g][:], wout_t[k][:, csl],
                    start=(k == 0), stop=False,
                )
            # bias: accumulate ones[128].T @ b_out[csl] (K=1 matmul)
            nc.tensor.matmul(
                p[:], ones_t[0:1, :], bout_sb[0:1, csl],
                start=False, stop=True,
            )
            o = outp.tile([128, CW], F32, tag="ostage")
            if ch % 2 == 0:
                nc.vector.tensor_copy(o[:], p[:])
            else:
                nc.scalar.copy(o[:], p[:])
            nc.sync.dma_start(out[g * 128:(g + 1) * 128, csl], o[:])

        cls_units = [(g, ch) for g in range(NG) for ch in range(NCH)]
        cls_done = 0

        # ---------------- recurrence ---------------------------------------
        h_prev = None  # A-layout [16, 512] f32 tile of h_t
        for t in range(T):
            gx_t = gxp.tile([BC, G3], BF16, tag="gxt")
            nc.sync.dma_start(
                gx_t[:], gx_d[t // 8][(t % 8) * BC:(t % 8 + 1) * BC, :])

            if t > 0:
                g_prev, s_prev = (t - 1) // 8, (t - 1) % 8
                p_gh = psA.tile([128, G3], F32, tag="gh")
                for nch in (0, 2, 1):
                    csl = slice(nch * 512, (nch + 1) * 512)
                    for k in range(KT):
                        nc.tensor.matmul(
                            p_gh[0:BC, csl],
                            hsT[k][g_prev][:, s_prev * 16:(s_prev + 1) * 16],
                            whh_t[k][:, csl],
                            start=(k == 0), stop=(k == KT - 1),
                        )

            # classifier units placed here, AFTER this step's gh matmuls in
            # the PE stream: the engine executes its stream in order, so
            # these fill the PE wait while DVE/ACT/GpSimd run the gate math
            if t >= 8:
                avail = 24 * (t // 8)
                target = min(avail, 3 * (t - 7))
                while cls_done < target:
                    cls_unit(*cls_units[cls_done])
                    cls_done += 1

            h_new = work.tile([BC, H], F32, tag="hA", bufs=2)

            def gslice(gate):
                return slice(gate * 512, (gate + 1) * 512)

            r = work.tile([BC, H], F32, tag="r", bufs=2, name="r")
            z = work.tile([BC, H], F32, tag="z", bufs=2, name="z")
            n = work.tile([BC, H], F32, tag="n", bufs=2, name="n")
            if t == 0:
                nc.scalar.activation(r[:], gx_t[:, gslice(0)], AF.Sigmoid)
                nc.scalar.activation(n[:], gx_t[:, gslice(2)], AF.Tanh)
                nc.scalar.activation(z[:], gx_t[:, gslice(1)], AF.Sigmoid)
                omz = work.tile([BC, H], F32, tag="omz", name="omz")
                nc.vector.tensor_scalar(
                    omz[:], z[:], -1.0, 1.0, op0=ALU.mult, op1=ALU.add)
                nc.vector.tensor_tensor(h_new[:], omz[:], n[:], op=ALU.mult)
            else:
                # Full-width [16,512] ops; chain r -> n -> z-tail with
                # h' = n + z*(h - n).  gh chunk order is (hr, hn, hz) so the
                # long r/n chains overlap the hz matmul; the z-tail after hz
                # is just zp -> sigmoid -> z*(h-n) -> add.
                # GpSimd cannot read PSUM: psum-touching ops stay on DVE.
                rp = work.tile([BC, H], F32, tag="rp", name="rp")
                nc.vector.tensor_tensor(
                    rp[:], p_gh[0:BC, gslice(0)], gx_t[:, gslice(0)],
                    op=ALU.add)
                nc.scalar.activation(r[:], rp[:], AF.Sigmoid)

                rhn = work.tile([BC, H], F32, tag="rhn", name="rhn")
                nc.vector.tensor_tensor(
                    rhn[:], r[:], p_gh[0:BC, gslice(2)], op=ALU.mult)
                nc.vector.tensor_tensor(
                    rhn[:], rhn[:], gx_t[:, gslice(2)], op=ALU.add)
                nc.scalar.activation(n[:], rhn[:], AF.Tanh)

                # z-tail in halves so sigmoid/multiply/add pipeline after
                # the hz matmul chunk lands
                for c in range(2):
                    hsl = slice(c * 256, (c + 1) * 256)
                    zsl = slice(512 + c * 256, 512 + (c + 1) * 256)
                    zp = work.tile([BC, 256], F32, tag=f"zp{c}", name=f"zp{c}")
                    nc.vector.tensor_tensor(
                        zp[:], p_gh[0:BC, zsl], gx_t[:, zsl], op=ALU.add)
                    nc.scalar.activation(z[:, hsl], zp[:], AF.Sigmoid)
                    hmn = work.tile([BC, 256], F32, tag=f"hmn{c}",
                                    name=f"hmn{c}")
                    nc.gpsimd.tensor_tensor(
                        hmn[:], h_prev[:, hsl], n[:, hsl], op=ALU.subtract)
                    zhmn = work.tile([BC, 256], F32, tag=f"zhmn{c}",
                                     name=f"zhmn{c}")
                    nc.vector.tensor_tensor(
                        zhmn[:], z[:, hsl], hmn[:], op=ALU.mult)
                    nc.vector.tensor_tensor(
                        h_new[:, hsl], n[:, hsl], zhmn[:], op=ALU.add)

            # transpose h_{t+1} into the bf16 hsT stash
            g, s = t // 8, t % 8
            p_tr = psC.tile([128, 64], F32, tag="tr")
            for hc in range(KT):
                nc.tensor.transpose(
                    p_tr[:, hc * 16:(hc + 1) * 16],
                    h_new[:, hc * 128:(hc + 1) * 128], id_t[:])
            for hc in range(KT):
                dst = hsT[hc][g][:, s * 16:(s + 1) * 16]
                srcap = p_tr[:, hc * 16:(hc + 1) * 16]
                if hc % 2 == 0:
                    nc.scalar.copy(dst, srcap)
                else:
                    nc.vector.tensor_copy(dst, srcap)
            h_prev = h_new

            if t == 0:
                nc.sync.dma_start(bout_sb[:], boutr[:])
                nc.vector.memset(ones_t[:], 1.0)
            elif 1 <= t <= 6:
                # spread the 96 wout chunk loads across early steps so they
                # never block the per-step gx prefetches on the DMA pipe
                emit_wout_dmas((t - 1) * 16, t * 16)


        while cls_done < len(cls_units):
            cls_unit(*cls_units[cls_done])
            cls_done += 1

    nc.compile()
    return nc


def _prep(inputs):
    img = np.asarray(inputs["img"], np.float32)
    cap = np.asarray(inputs["cap"], np.int64)
    emb = np.asarray(inputs["emb"], np.float32)
    W_ih = np.asarray(inputs["W_ih"], np.float32)
    W_hh = np.asarray(inputs["W_hh"], np.float32)
    W_out = np.asarray(inputs["W_out"], np.float32)
    b_out = np.asarray(inputs["b_out"], np.float32)
    # b_ih / b_hh are structurally zero in this problem's setup_inputs.

    word = emb[cap[:, :-1]]                       # [B, T-1, E]
    x = np.concatenate([img[:, None, :], word], axis=1)  # [B, T, E]

    wihT = np.ascontiguousarray(W_ih.T).astype(ml_dtypes.bfloat16)
    whhT = np.ascontiguousarray(W_hh.T).astype(ml_dtypes.bfloat16)
    woutT = np.ascontiguousarray(W_out.T).astype(ml_dtypes.bfloat16)
    boutr = np.ascontiguousarray(
        b_out.reshape(1, V).astype(ml_dtypes.bfloat16))
    id16 = np.eye(16, dtype=np.float32)

    in_maps = []
    for c in range(NCORES):
        xc = x[c * BC:(c + 1) * BC]               # [16, T, E]
        xTc = np.ascontiguousarray(
            xc.transpose(2, 1, 0).reshape(E, R)).astype(ml_dtypes.bfloat16)
        in_maps.append({
            "xT": xTc, "wihT": wihT, "whhT": whhT, "woutT": woutT,
            "boutr": boutr, "ident": id16,
        })
    return in_maps


def run_spmd(in_maps):
    """Compile (cached) + execute the SPMD program; returns BassKernelResults."""
    if "nc" not in _CACHE:
        _CACHE["nc"] = _build()
    return run_bass_kernel_spmd(_CACHE["nc"], in_maps, list(range(NCORES)))


def kernel(**inputs):
    global LAST_RESULTS
    in_maps = _prep(inputs)
    res = run_spmd(in_maps)
    LAST_RESULTS = res
    logits = np.empty((B, T, V), np.float32)
    for c in range(NCORES):
        o = res.results[c]["out"]                 # [R, V], t-major rows
        logits[c * BC:(c + 1) * BC] = o.reshape(T, BC, V).transpose(1, 0, 2)
    return logits
